# revision 48
# baseline (speedup 1.0000x reference)
"""AttentionPooling Trainium2 kernel (8 NeuronCores, SPMD).

Reference computation:
    scores = tanh(x @ W1 + b1) @ W2          # [N, 4]
    w      = segment_softmax(scores, batch)  # per-graph softmax over nodes
    out[g] = mean_h( sum_{n in g} w[n,h] * x[n] )   # [G, 256]

Sharding: 64 graphs per core (512 graphs / 8 cores), LPT-bin-packed into
octs of 8 graphs so the largest oct is minimal; each oct's nodes are padded
to a fixed number of 128-node tiles (T) so every core runs the identical
instruction stream.  Weights are replicated; per-graph outputs are disjoint,
so the host concatenates the 8 core outputs and undoes the LPT permutation.

On-core algorithm (single pass over x; 1024-node chunks; all DMA issued
on one SP queue so transfer order tracks consumption order; single packed
constants DMA read through bitcast views):
  - two streams: xnm (node-major packed rows [x | 1 | bloc], bf16, feeds
    the pooling matmul at full precision) and xt8 (fp8-e4m3
    host-transposed x^T in DoubleRow k-tile layout, feeds the score MLP,
    prefetched 2 chunks ahead of the matching xnm).
  - z^T = W1^T @ x^T on TensorE as fp8 DoubleRow matmuls (both 128-row
    k-tiles contracted per pass at 0.5 cyc/row); W1 is quantized to fp8
    plus an fp8 *residual* matrix accumulated in a second matmul, which
    removes the W1 quantization error (final rel-err ~1.1e-2, dominated
    by the x quantization, vs the 2e-2 tolerance).
  - tanh(+per-partition bias) on ScalarE: one contiguous 1024-elem
    instruction per 128-row H_out chunk (per-ko PSUM tiles), amortizing
    the ScalarE access-latency tax; one exp per chunk PAIR.
  - the whole schedule is software-pipelined 3 deep with per-engine
    emission in readiness order (PE: MLP_k0(c) | pool(c-3) | MLP_k1(c) |
    s(c-1); ScalarE: tanh_k0(c) | exp-pair | tanh_k1(c)) so the 4-deep
    in-order wait queues never park ready work behind stalled work.
  - s scores: per 128-node tile, two K=128 matmuls (stationary = t^T
    columns, ldweights free), psum group closed per tile.
  - oct one-hot mask on DVE from the bloc column via a broadcast-AP
    iota-compare; sel[128, 8*32] = e (broadcast over 8 oct slots) * mask
  - pooled[(oct%4)*32 + slot*4 + head, 0:256] += sel_j^T @ [x_j | 1],
    accumulated across the whole shard in 2 persistent PSUM banks;
    column 256 (the ones column) gives the softmax denominator
  - epilogue: one DVE divide by the denominator column, average heads
    with a constant matmul written back into the pool bank, DMA out
    [64, 256] fp32 per core; poolA's epilogue (octs 0-3, complete at
    ~chunk 12) overlaps the main loop.
"""

import numpy as np
import ml_dtypes

BF16 = ml_dtypes.bfloat16
E4M3 = ml_dtypes.float8_e4m3fn

N_CORES = 8
H = 256
HEADS = 4
GRP = 8  # graphs per oct group
SELW = GRP * HEADS  # 32 selector columns per node
ROW = H + 2  # packed row: x(256) | ones(1) | bloc(1)
BLOC = H + 1  # bloc column index
CST_BYTES = 1176  # packed consts: w18(512)|r18(512)|w2(16)|b1(8)|shs(64)|iot(64)

_NC_CACHE = {}
LAST_RESULT = None


def _build_nc(T: int, n_grps: int, repeats: int = 1):
    """Build the SPMD Bass program. T = 128-node tiles per oct group."""
    import concourse.bacc as bacc
    import concourse.mybir as mybir
    from concourse.tile import TileContext

    fp32 = mybir.dt.float32
    bf16 = mybir.dt.bfloat16
    fp8 = mybir.dt.float8e4
    AF = mybir.ActivationFunctionType
    DR = mybir.MatmulPerfMode.DoubleRow

    n_tiles = n_grps * T
    assert n_tiles % 8 == 0
    n_chunks = n_tiles // 8  # 1024-node chunks
    assert n_grps == 8, "psum layout assumes 8 octs (64 graphs) per core"

    nc = bacc.Bacc(trn_type="TRN2")

    uint8 = mybir.dt.uint8
    xnm = nc.dram_tensor("xnm", [n_chunks, 128, 8 * ROW], bf16, kind="ExternalInput")
    xt8 = nc.dram_tensor("xt8", [n_chunks, 128, 2048], fp8, kind="ExternalInput")
    # all constants packed into one DMA: w18|r18|w2|b1|shs|iot (byte offsets)
    cst = nc.dram_tensor("cst", [128, CST_BYTES], uint8, kind="ExternalInput")
    out = nc.dram_tensor("out", [64, H], fp32, kind="ExternalOutput")

    with TileContext(nc, pool_alloc_mode="queue") as tc:
        with (
            tc.tile_pool(name="consts", bufs=1) as cpool,
            tc.tile_pool(name="acc", bufs=1, space="PSUM") as acc_pool,
        ):
            cst_sb = cpool.tile([128, CST_BYTES], uint8)
            nc.sync.dma_start(cst_sb[:], cst.ap())
            # dummy activation with no dependencies: hoists the (1.3us)
            # LoadActFuncSet to t=0 instead of just before the first tanh
            dmy = cpool.tile([1, 2], bf16)
            nc.vector.memset(dmy[:], 0)
            nc.scalar.activation(dmy[:], dmy[:], AF.Exp)
            w18_sb = cst_sb[:, 0:512].bitcast(fp8)
            r18_sb = cst_sb[:, 512:1024].bitcast(fp8)
            w2_sb = cst_sb[:, 1024:1040].bitcast(bf16)
            b1_sb = cst_sb[:, 1040:1048].bitcast(fp32)
            shs_sb = cst_sb[:, 1048:1112].bitcast(bf16)
            iot_sb = cst_sb[:, 1112:1176].bitcast(bf16)

            # persistent accumulators: rows = (oct%4)*32 + jj*4 + h, col 256 = seg_e
            poolA = acc_pool.tile([128, H + 1], fp32)
            poolB = acc_pool.tile([128, H + 1], fp32)

            with (
                tc.tile_pool(name="data", bufs=9) as dpool,
                tc.tile_pool(name="work", bufs=3) as wpool,
                tc.tile_pool(name="ep", bufs=1) as ep,
                tc.tile_pool(name="zp", bufs=1, space="PSUM") as zpool,
                tc.tile_pool(name="sp", bufs=2, space="PSUM") as spool,
            ):

                ep_state = {}

                def epilogue_norm(ps, idx, r0=0, r1=128, tag=""):
                    """normalize rows [r0:r1] by seg_e (DVE epilogue phase).
                    (tensor_scalar divide is not a valid DVE ISA op, so
                    clamp -> reciprocal -> multiply.)"""
                    nr = r1 - r0
                    seg = ep.tile([nr, 1], fp32, name=f"seg{idx}{tag}")
                    nc.vector.tensor_scalar(
                        seg[:], ps[r0:r1, H : H + 1], 1e-30, None,
                        mybir.AluOpType.max,
                    )
                    rec = ep.tile([nr, 1], fp32, name=f"rec{idx}{tag}")
                    nc.vector.reciprocal(rec[:], seg[:])
                    if idx not in ep_state:
                        ep_state[idx] = ep.tile([128, H], bf16, name=f"norm{idx}")
                    norm = ep_state[idx]
                    nc.vector.tensor_scalar(
                        norm[r0:r1, :], ps[r0:r1, 0:H], rec[:], None,
                        mybir.AluOpType.mult,
                    )

                def epilogue_mm(ps, idx, r0, r1, start, stop):
                    """partial head-mean matmul (contraction rows r0:r1),
                    accumulated into the pool psum bank (complete by now)."""
                    norm = ep_state[idx]
                    nc.tensor.matmul(
                        ps[0:32, 0:H], shs_sb[r0:r1, :], norm[r0:r1, :],
                        start=start, stop=stop,
                    )

                def epilogue_out(ps, idx):
                    fin = ps[0:32, 0:H]
                    osb = ep.tile([32, H], fp32, name=f"osb{idx}")
                    nc.vector.tensor_copy(osb[:], fin)  # DVE is idle at the tail
                    nc.sync.dma_start(out.ap()[idx * 32 : (idx + 1) * 32, :], osb[:])

                xt8_all = xt8.ap()
                xnm_all = xnm.ap()
                w18_v = w18_sb.rearrange("p (ko kt m) -> p ko kt m", ko=2, kt=2)
                r18_v = r18_sb.rearrange("p (ko kt m) -> p ko kt m", ko=2, kt=2)

                # Software-pipelined schedule, 2 chunks deep.  Per-engine
                # instruction order matches readiness time so the 4-deep
                # in-order wait queues never park ready work behind stalled
                # work:  PE gets  MLP_k0(c) | s_k0(c-1) | pool(c-2) |
                # MLP_k1(c) | s_k1(c-1),  ScalarE gets  tanh_k0(c) |
                # exp(c-1) | tanh_k1(c).
                st = {}  # ch -> state dict
                xt8_tiles = {}  # prefetched one iteration ahead on the SWDGE queue

                def fetch_xt8(it2):
                    t = dpool.tile([128, 2048], fp8, name="xt8_sb")
                    nc.sync.dma_start(t[:], xt8_all[it2 % n_chunks])
                    xt8_tiles[it2] = t

                n_iters = repeats * n_chunks
                for it in range(n_iters + 3):
                    cur = it if it < n_iters else None
                    pv = it - 1 if 0 <= it - 1 < n_iters else None
                    pp = it - 3 if 0 <= it - 3 < n_iters else None

                    if cur is not None:
                        ch = cur % n_chunks
                        s = st[cur] = {}
                        if cur == 0:
                            for k in range(min(2, n_iters)):
                                fetch_xt8(k)
                        if cur + 2 < n_iters:
                            fetch_xt8(cur + 2)
                        s["xnm"] = dpool.tile([128, 8 * ROW], bf16, name="xnm_sb")
                        nc.sync.dma_start(s["xnm"][:], xnm_all[ch])
                        xt8_sb = xt8_tiles.pop(cur)
                        xt8_v = xt8_sb[:].rearrange("p (kt n) -> p kt n", kt=2)
                        s["tt"] = wpool.tile([128, 2048], bf16, name="tt", tag="tt")
                        # s-score psum is shared by iteration PAIRS (one exp
                        # per pair amortizes ScalarE access latency)
                        if cur % 2 == 0:
                            s["sps"] = spool.tile(
                                [128, 16 * HEADS], fp32, name="s_ps", tag="s_ps"
                            )
                        else:
                            s["sps"] = st[cur - 1]["sps"]

                        def mlp(ko, s=s, xt8_v=xt8_v):
                            # z^T = W1^T @ x^T: fp8 DoubleRow (both k-tiles per
                            # pass); the fp8 residual R1 removes the W1
                            # quantization error.  ztk[ko] holds H_out chunk ko
                            # for all 1024 nodes -> one contiguous tanh each.
                            zt = zpool.tile(
                                [128, 1024], fp32, name=f"ztk{ko}", tag=f"ztk{ko}"
                            )
                            s[f"zt{ko}"] = zt
                            for s2 in range(2):
                                dst = zt[:, s2 * 512 : (s2 + 1) * 512]
                                rhs = xt8_v[:, :, s2 * 512 : (s2 + 1) * 512]
                                nc.tensor.matmul(
                                    dst, w18_v[:, ko], rhs,
                                    start=True, stop=False, perf_mode=DR,
                                )
                                nc.tensor.matmul(
                                    dst, r18_v[:, ko], rhs,
                                    start=False, stop=True, perf_mode=DR,
                                )

                        def tanh(ko, s=s):
                            nc.scalar.activation(
                                s["tt"][:, ko * 1024 : (ko + 1) * 1024],
                                s[f"zt{ko}"][:],
                                AF.Tanh,
                                bias=b1_sb[:, ko : ko + 1],
                            )

                        s["mlp"], s["tanh"] = mlp, tanh

                    def s_mm(c, s):
                        # s scores: per 128-node tile, stationary = t^T cols
                        # (ldweights are free), K=128 per H_out chunk.  Each
                        # j's psum group closes (ko1 stop) before the next
                        # opens: one open group per psum (partition, bank).
                        off = (c % 2) * 8 * HEADS
                        for j in range(8):
                            for ko in range(2):
                                nc.tensor.matmul(
                                    s["sps"][
                                        :, off + j * HEADS : off + (j + 1) * HEADS
                                    ],
                                    s["tt"][
                                        :,
                                        ko * 1024 + j * 128 : ko * 1024
                                        + j * 128
                                        + 128,
                                    ],
                                    w2_sb[:, ko * HEADS : (ko + 1) * HEADS],
                                    start=(ko == 0),
                                    stop=(ko == 1),
                                )

                    # --- PE stream (readiness order) ---
                    if cur is not None:
                        st[cur]["mlp"](0)
                    if pp is not None:
                        # pooled[(o%4)*32 : +32, :] += sel_j^T @ [x_j | 1]
                        sp = st[pp]
                        ch2 = pp % n_chunks
                        for j in range(8):
                            t_glob = ch2 * 8 + j
                            o = t_glob // T
                            tau = t_glob % T
                            ps = poolA if (o % 8) < 4 else poolB
                            r0 = (o % 4) * 32
                            nc.tensor.matmul(
                                ps[r0 : r0 + 32, :],
                                sp["sel"][:, j * SELW : (j + 1) * SELW],
                                sp["xnm"][:, j * ROW : j * ROW + H + 1],
                                start=(tau == 0),
                                stop=(tau == T - 1),
                                tile_position=(0, r0),
                            )
                        del st[pp]
                        # poolA (octs 0-3) is complete once tile 4T-1 has
                        # pooled -> run its epilogue under the main loop
                        base_it = (repeats - 1) * n_chunks
                        if pp == base_it + (4 * T - 1) // 8:
                            epilogue_norm(poolA, 0)  # octs 0-3 done
                        elif pp == base_it + (4 * T - 1) // 8 + 1:
                            epilogue_mm(poolA, 0, 0, 128, True, True)
                        elif pp == base_it + (4 * T - 1) // 8 + 2:
                            epilogue_out(poolA, 0)
                    if cur is not None:
                        st[cur]["mlp"](1)
                    if pv is not None:
                        s_mm(pv, st[pv])

                    # --- ScalarE stream ---
                    if cur is not None:
                        st[cur]["tanh"](0)
                    if pv is not None and (pv % 2 == 1 or pv == n_iters - 1):
                        # one exp per iteration pair (or lone final chunk)
                        sp = st[pv]
                        npair = 2 if pv % 2 == 1 else 1
                        width = npair * 8 * HEADS
                        e_sb = wpool.tile([128, 16 * HEADS], bf16, name="e_sb")
                        nc.scalar.activation(
                            e_sb[:, 0:width], sp["sps"][:, 0:width], AF.Exp
                        )
                        for half in range(npair):
                            st[pv - (npair - 1) + half]["e"] = e_sb[
                                :, half * 8 * HEADS : (half + 1) * 8 * HEADS
                            ]
                    if cur is not None:
                        st[cur]["tanh"](1)

                    # --- DVE stream: mask+sel for every chunk whose e just
                    # arrived (pair granularity) ---
                    if pv is not None and (pv % 2 == 1 or pv == n_iters - 1):
                        first = pv - 1 if pv % 2 == 1 else pv
                        for c in range(first, pv + 1):
                            sp = st[c]
                            # oct one-hot masks: (bloc == iota), broadcast op
                            mk = wpool.tile([128, 8 * SELW], bf16, name="mk")
                            bloc_b = (
                                sp["xnm"][:]
                                .rearrange("p (j c) -> p j c", j=8)[
                                    :, :, BLOC : BLOC + 1
                                ]
                                .broadcast_to((128, 8, SELW))
                            )
                            iot_b = (
                                iot_sb
                                .rearrange("p (o c) -> p o c", o=1)
                                .broadcast_to((128, 8, SELW))
                            )
                            nc.vector.tensor_tensor(
                                mk[:].rearrange("p (j c) -> p j c", j=8),
                                bloc_b,
                                iot_b,
                                mybir.AluOpType.is_equal,
                            )
                            # selector = e * mask (e broadcast over oct slots)
                            sp["sel"] = wpool.tile(
                                [128, 8 * SELW], bf16, name="sel"
                            )
                            e_b = (
                                sp["e"]
                                .rearrange("p (j o h) -> p j o h", j=8, o=1)
                                .broadcast_to((128, 8, GRP, HEADS))
                            )
                            sel_v = sp["sel"][:].rearrange(
                                "p (j o h) -> p j o h", j=8, o=GRP
                            )
                            mk_v = mk[:].rearrange(
                                "p (j o h) -> p j o h", j=8, o=GRP
                            )
                            if c == n_iters - 1:
                                # tail chunk: two halves so pooling overlaps
                                for hf in range(2):
                                    nc.vector.tensor_tensor(
                                        sel_v[:, hf * 4 : (hf + 1) * 4],
                                        e_b[:, hf * 4 : (hf + 1) * 4],
                                        mk_v[:, hf * 4 : (hf + 1) * 4],
                                        mybir.AluOpType.mult,
                                    )
                            else:
                                nc.vector.tensor_tensor(
                                    sel_v, e_b, mk_v, mybir.AluOpType.mult
                                )

                epilogue_norm(poolB, 1)
                epilogue_mm(poolB, 1, 0, 128, True, True)
                epilogue_out(poolB, 1)

    nc.finalize()
    return nc


def _lpt_octs(counts, n_octs):
    """LPT-pack graphs into octs of GRP graphs, minimizing the max oct size."""
    import heapq

    order = np.argsort(-counts)
    heap = [(0, i, []) for i in range(n_octs)]
    heapq.heapify(heap)
    for g in order:
        popped = []
        while True:
            sz, i, lst = heapq.heappop(heap)
            if len(lst) < GRP:
                break
            popped.append((sz, i, lst))
        heapq.heappush(heap, (sz + int(counts[g]), i, lst + [int(g)]))
        for p in popped:
            heapq.heappush(heap, p)
    octs = [None] * n_octs
    for sz, i, lst in heap:
        octs[i] = lst
    return octs


def _host_prep(x, batch, W1, b1, W2, G):
    """Shard + pad inputs; build all per-core DRAM arrays."""
    gpc = G // N_CORES  # graphs per core
    n_grps = gpc // GRP  # oct groups per core
    counts = np.bincount(batch, minlength=G)
    octs = _lpt_octs(counts, G // GRP)  # balanced graph -> oct assignment
    oct_sums = np.array([counts[o].sum() for o in octs])
    T = int(np.ceil(oct_sums.max() / 128))
    # 1024-node chunks hold 8 tiles; n_grps == 8 makes n_tiles a multiple of 8
    grp_nodes = T * 128
    n_pad = n_grps * grp_nodes

    starts = np.zeros(G + 1, dtype=np.int64)
    np.cumsum(counts, out=starts[1:])

    # output row (o*GRP + jj) holds graph octs[o][jj]
    gmap = np.array([g for o in octs for g in o], dtype=np.int64)

    x_bf = x.astype(BF16)
    xam = np.zeros((N_CORES, n_pad, ROW), dtype=BF16)
    for c in range(N_CORES):
        xam[c, :, BLOC] = BF16(-1.0)  # padding nodes match no oct slot
    for c in range(N_CORES):
        for gl in range(n_grps):
            o = c * n_grps + gl
            base = gl * grp_nodes
            pos = base
            for jj, g in enumerate(octs[o]):
                s, e = int(starts[g]), int(starts[g + 1])
                cnt = e - s
                xam[c, pos : pos + cnt, 0:H] = x_bf[s:e]
                xam[c, pos : pos + cnt, H] = BF16(1.0)
                xam[c, pos : pos + cnt, BLOC] = BF16(jj)
                pos += cnt

    n_chunks = n_pad // 1024
    # node-major chunk layout: one contiguous 4160B read per partition/chunk
    xam2 = np.ascontiguousarray(
        xam.reshape(N_CORES, n_chunks, 8, 128, ROW)
        .transpose(0, 1, 3, 2, 4)
        .reshape(N_CORES, n_chunks, 128, 8 * ROW)
    )

    # fp8 transposed stream in DoubleRow layout:
    # xt8[c, ch, p, kt*1024 + n] = fp8(x_pad[ch*1024+n, kt*128+p])
    x8 = xam[:, :, 0:H].astype(E4M3)  # [cores, n_pad, 256]
    xt8 = np.ascontiguousarray(
        x8.reshape(N_CORES, n_chunks, 1024, 2, 128)
        .transpose(0, 1, 4, 3, 2)
        .reshape(N_CORES, n_chunks, 128, 2048)
    )

    # W1 fp8 + fp8 residual, DoubleRow stationary layout:
    # w18[p, ko*256 + kt*128 + m] = fp8(W1[kt*128+p, ko*128+m])
    W18 = W1.astype(E4M3)
    R1 = (W1 - W18.astype(np.float32)).astype(E4M3)
    def _wlay(w):
        out = np.zeros((128, 512), dtype=E4M3)
        for ko in range(2):
            for kt in range(2):
                out[:, ko * 256 + kt * 128 : ko * 256 + kt * 128 + 128] = w[
                    kt * 128 : (kt + 1) * 128, ko * 128 : (ko + 1) * 128
                ]
        return out
    w18h = _wlay(W18)
    r18h = _wlay(R1)

    # w2 chunks: [:, ko*HEADS : +HEADS] = W2[ko*128:(ko+1)*128, :]
    w2h = np.zeros((128, 2 * HEADS), dtype=BF16)
    for ko in range(2):
        w2h[:, ko * HEADS : (ko + 1) * HEADS] = W2[
            ko * 128 : (ko + 1) * 128, :
        ].astype(BF16)
    b1h = np.stack([b1[0:128], b1[128:256]], axis=1).astype(np.float32)  # [128, 2]
    # head-mean matrix: rows p=(o%4)*32+jj*4+h -> graph column p//4, value 1/4
    shsh = np.zeros((128, 32), dtype=BF16)
    shsh[np.arange(128), np.arange(128) // HEADS] = BF16(0.25)
    # iota over oct slots, one value per selector column, bcast to all partitions
    ioth = np.broadcast_to(
        (np.arange(SELW) // HEADS).astype(BF16)[None, :], (128, SELW)
    ).copy()

    # pack all constants into one [128, CST_BYTES] uint8 blob (single DMA)
    csth = np.concatenate(
        [
            np.ascontiguousarray(a).view(np.uint8).reshape(128, -1)
            for a in (w18h, r18h, w2h, b1h, shsh, ioth)
        ],
        axis=1,
    )
    assert csth.shape == (128, CST_BYTES), csth.shape

    return T, n_grps, xam2, xt8, csth, gmap


def kernel(x, batch, W1, b1, W2, num_graphs):
    global LAST_RESULT
    from concourse.bass_utils import run_bass_kernel_spmd

    x = np.asarray(x, dtype=np.float32)
    batch = np.asarray(batch).astype(np.int64)
    W1 = np.asarray(W1, dtype=np.float32)
    b1 = np.asarray(b1, dtype=np.float32)
    W2 = np.asarray(W2, dtype=np.float32)
    G = int(num_graphs)

    T, n_grps, xam, xt8, csth, gmap = _host_prep(x, batch, W1, b1, W2, G)

    key = (T, n_grps)
    if key not in _NC_CACHE:
        _NC_CACHE[key] = _build_nc(T, n_grps)
    nc = _NC_CACHE[key]

    in_maps = [
        {"xnm": xam[c], "xt8": xt8[c], "cst": csth} for c in range(N_CORES)
    ]

    res = run_bass_kernel_spmd(nc, in_maps, core_ids=list(range(N_CORES)))
    LAST_RESULT = res
    raw = np.concatenate([res.results[c]["out"] for c in range(N_CORES)], axis=0)
    out = np.empty_like(raw)
    out[gmap] = raw  # undo the LPT graph permutation
    return out
